# revision 23
# baseline (speedup 1.0000x reference)
"""Trainium2 Bass kernel for ContextHyperLinearSSM.

Computes out[b,:] = x[b,:] @ (WA[context[b]] * adj_xx) + u[b,:] @ (WB[context[b]] * adj_xu)

Strategy: shard the CONTEXT axis across the 8 cores.  The host sorts
contexts by sample count (descending) and deals them round-robin, so
rank j on every core has a similar count and one SPMD program (with a
per-rank padded group size G_j baked in at build time) fits all cores.
Each core streams its 64 contexts' weight banks from HBM exactly once
in bf16, applies the adjacency masks on-device, and runs 3 accumulating
matmuls per context (f32 PSUM).  Each sample's row is computed by
exactly one core, so the host-side unshard is a pure scatter.

Device-side layout: contexts are processed in groups; a group's payload
(weights + gathered/transposed x,u activations for its contexts) is
packed by the host into one contiguous HBM slab so a single DMA per
group moves 128 x ~13KB descriptors at full efficiency.  One in-place
DVE multiply per half-group masks the weights against a combined
[adjB|adjA] bf16 tile.  A group's contexts accumulate into PSUM banks
(64-aligned partition slots x 2 free halves); ACT/Pool drain the banks
with f32->bf16 cast copies.  The group schedule tapers ([8]*7+[4,2,2])
so the post-DMA pipeline tail is only a 2-context chain.
"""

import sys

sys.path.insert(0, "/opt/trn_rl_repo")

import ml_dtypes
import numpy as np

import concourse.bass as bass
import concourse.mybir as mybir
import concourse.tile as tile
from concourse import bacc
from concourse.bass_utils import run_bass_kernel_spmd

N_CORES = 8
SCHED = (8, 8, 8, 8, 8, 8, 8, 8)   # contexts per group (sum = 64)
W_BUFS = 5

# blob / matmul operand dtype: bf16 halves HBM traffic and streams the PE
# at 4x the f32 rate; PSUM still accumulates in f32 (rel err ~3e-3 << 2e-2)
IO_DT = mybir.dt.bfloat16


def _install_profile_shim():
    """Register the NTFF profile hook that trn_boot skips when
    antenv.axon_hooks is missing from the image (profiling only)."""
    import types
    if "antenv.axon_hooks" in sys.modules:
        return
    try:
        from trn_agent_boot.trn_boot import _ntff_profile_via_ctypes
        hook = _ntff_profile_via_ctypes("/opt/axon/libaxon_pjrt.so")
    except Exception:
        hook = None
    mod = types.ModuleType("antenv.axon_hooks")
    mod.get_axon_ntff_profile_hook = lambda: hook
    mod.set_axon_ntff_profile_hook = lambda h: None
    sys.modules["antenv.axon_hooks"] = mod


def _blocks(g):
    """Context blocks of group g: [W_blk | A_blk] slabs in the blob.
    Two half-blocks per group; one DMA covers the whole slab except for
    the LAST group, whose halves stream as separate DMAs so the final
    mask->matmul->drain chain overlaps the stream tail.  (Finer splits
    don't pay: DMA lines under ~6KB lose HBM efficiency.)"""
    CT = SCHED[g]
    CH = CT // 2
    return [list(range(CH)), list(range(CH, CT))]


def _layout(S, Gs):
    """Static blob/psum/output layout for per-rank group sizes Gs[64].

    Per group g the HBM slab holds, per partition line:
      [W_h0 (CH*K*S) | A_h0 (K*sum G) | W_h1 | A_h1]
    Output slabs: per (g, sl) a block of R_gsl = Gs[rank of c=sl*2] rows;
    CT=8 groups write [R, 2*2*S] rows to "outA", smaller ones [R, 2*S]
    rows to "outB".
    """
    HS = S // 128
    K = HS + 1
    NG = len(SCHED)
    srank = np.concatenate([[0], np.cumsum(SCHED)]).astype(int)
    goff, aoffs, woffs, lws = [], {}, {}, []
    col = 0
    for g, CT in enumerate(SCHED):
        goff.append(col)
        w = 0
        for b, blk in enumerate(_blocks(g)):
            woffs[(g, b)] = w
            w += len(blk) * K * S
            for c in blk:
                j = srank[g] + c
                for kk in range(K):
                    aoffs[(j, kk)] = w
                    w += Gs[j]
        lws.append(w)
        col += w
    # output row offsets per (g, sl) into outA (CT=8) / outB (CT<8)
    rowoff = {}
    rA = rB = 0
    for g, CT in enumerate(SCHED):
        for sl in range(2):
            if CT <= sl * 2:
                continue
            R = Gs[srank[g] + sl * 2]
            if CT == 8:
                rowoff[(g, sl)] = ("outA", rA)
                rA += R
            else:
                rowoff[(g, sl)] = ("outB", rB)
                rB += R
    return K, NG, srank, goff, woffs, aoffs, lws, rowoff, rA, rB


def _build_program(S, A, Gs):
    """Build the per-core Bass program. Gs = per-rank padded group sizes."""
    f32 = mybir.dt.float32
    nc = bacc.Bacc("TRN2", target_bir_lowering=False)

    assert S % 128 == 0 and A == 128
    K, NG, srank, goff, woffs, aoffs, lws, rowoff, rA, rB = _layout(S, Gs)
    LWmax = max(lws)
    TOT = sum(lws)
    FF = 2   # contexts along the free dim of a PSUM bank (512 f32 / S)

    blob = nc.dram_tensor("blob", [128, TOT], IO_DT,
                          kind="ExternalInput").ap()
    adjm = nc.dram_tensor("adjm", [128, K * S], IO_DT,
                          kind="ExternalInput").ap()
    outA = nc.dram_tensor("outA", [max(rA, 1), 2 * FF * S], IO_DT,
                          kind="ExternalOutput").ap()
    outB = nc.dram_tensor("outB", [max(rB, 1), FF * S], IO_DT,
                          kind="ExternalOutput").ap()

    with tile.TileContext(nc) as tc:
        with (
            tc.tile_pool(name="const", bufs=1) as const,
            tc.tile_pool(name="w", bufs=W_BUFS) as wpool,
            tc.tile_pool(name="o", bufs=3) as opool,
            tc.tile_pool(name="psum", bufs=8, space="PSUM") as psum,
        ):
            # combined [adjB | adjA] mask, host-prepared bf16 lines
            adjC = const.tile([128, K * S], IO_DT)
            nc.sync.dma_start(adjC[:], adjm[:])
            adjC_b = {ch: adjC[:, None, :].to_broadcast([128, ch, K * S])
                      for ch in (1, 2, 4)}

            for g, CT in enumerate(SCHED):
                NB = -(-CT // 4)   # PSUM banks for this group
                blocks = _blocks(g)
                gt = wpool.tile([128, LWmax], IO_DT, tag="gb",
                                name=f"gb_{g}")
                if g < len(SCHED) - 1:
                    # one slab DMA: widest descriptors, peak HBM efficiency
                    nc.sync.dma_start(gt[:, :lws[g]],
                                      blob[:, goff[g]:goff[g] + lws[g]])
                else:
                    # last group: per-half DMAs overlap the stream tail
                    mid = woffs[(g, 1)]
                    nc.sync.dma_start(gt[:, :mid],
                                      blob[:, goff[g]:goff[g] + mid])
                    nc.sync.dma_start(gt[:, mid:lws[g]],
                                      blob[:, goff[g] + mid:goff[g] + lws[g]])
                ps_tiles = [psum.tile([128, FF * S], f32, tag="ps",
                                      name=f"ps_{g}_{t}")
                            for t in range(NB)]
                for b, blk in enumerate(blocks):
                    woff = woffs[(g, b)]
                    # mask this block's B+A weights, in <=2-context pieces
                    # so each context's matmuls start as soon as its slice
                    # is masked (shorter DVE->PE chain at the pipeline tail)
                    PC = min(2, len(blk))
                    for pi, p0 in enumerate(range(0, len(blk), PC)):
                        wvp = gt[:, woff + p0 * K * S:
                                 woff + (p0 + PC) * K * S].rearrange(
                            "p (c ks) -> p c ks", c=PC)
                        # alternate pieces between DVE and Pool so neither
                        # engine's queue gates the matmul stream
                        eng = nc.gpsimd if pi % 2 == 0 else nc.vector
                        eng.tensor_tensor(wvp, wvp, adjC_b[PC],
                                          mybir.AluOpType.mult)
                    for ci, c in enumerate(blk):
                        j = srank[g] + c
                        Gj = Gs[j]
                        t, r2 = divmod(c, 4)
                        sl, cf = divmod(r2, 2)
                        pslice = ps_tiles[t][sl * 64:sl * 64 + Gj,
                                             cf * S:cf * S + S]
                        for kk in range(K):
                            ao = aoffs[(j, kk)]
                            wo = woff + (ci * K + kk) * S
                            nc.tensor.matmul(
                                pslice,
                                lhsT=gt[:, ao:ao + Gj],
                                rhs=gt[:, wo:wo + S],
                                start=(kk == 0), stop=(kk == K - 1))
                out_sb = opool.tile([128, NB, FF, S], IO_DT,
                                    tag=f"ob{NB}", name=f"ob_{g}")
                for t in range(NB):
                    for sl in range(2):
                        if CT <= t * 4 + sl * 2:
                            continue
                        R = Gs[srank[g] + t * 4 + sl * 2]
                        dst = out_sb[sl * 64:sl * 64 + R, t].rearrange(
                            "p f s -> p (f s)")
                        src = ps_tiles[t][sl * 64:sl * 64 + R, :]
                        # all drains on ACT: a drain on the DVE queue would
                        # make the next group's mask multiply wait behind
                        # this group's matmul completion
                        nc.scalar.copy(dst, src)
                for sl in range(2):
                    if (g, sl) not in rowoff:
                        continue
                    R = Gs[srank[g] + sl * 2]
                    which, ro = rowoff[(g, sl)]
                    dst = (outA if which == "outA" else outB)[ro:ro + R, :]
                    src = out_sb[sl * 64:sl * 64 + R].rearrange(
                        "p t f s -> p (t f s)")
                    # NOTE: issue on Scalar only — an out-DMA issue on Sync
                    # would wait for this group's drain on the Sync queue and
                    # block every later blob-DMA issue behind it.
                    nc.scalar.dma_start(dst, src)

    nc.compile()
    return nc


def kernel(x, u, WA, WB, adj_xx, adj_xu, context, _trace=False):
    B, S = x.shape
    _, A = u.shape
    C = WA.shape[0]
    assert C % N_CORES == 0
    CP = C // N_CORES
    HS = S // 128
    K = HS + 1

    # ---- host-side shard: sort contexts by count, deal round-robin ----
    context = np.asarray(context)
    cnt = np.bincount(context, minlength=C)
    order_desc = np.argsort(-cnt, kind="stable")        # context ids
    # padded group size per per-core rank j: max count among the 8 cores
    # at that rank (= the first of the slice, counts sorted desc)
    Gs = [max(2, int(-2 * (-cnt[order_desc[j * N_CORES]] // 2)))
          for j in range(CP)]
    assert max(Gs) <= 64 and sum(SCHED) == CP

    order = np.argsort(context, kind="stable")
    starts = np.zeros(C + 1, np.int64)
    starts[1:] = np.cumsum(cnt)

    Xf = np.asarray(x, np.float32)
    Uf = np.asarray(u, np.float32)
    WA = np.ascontiguousarray(WA, np.float32)
    WB = np.ascontiguousarray(WB, np.float32)

    K_, NG, srank, goff, woffs, aoffs, lws, rowoff, rA, rB = _layout(S, Gs)

    # combined mask lines: [adj_xu[p,:] | adj_xx[p,:] (chunk0) | ...]
    adjm = np.empty((128, K * S), np.float32)
    adjm[:, :S] = np.asarray(adj_xu, np.float32)
    for h in range(HS):
        adjm[:, (1 + h) * S:(2 + h) * S] = \
            np.asarray(adj_xx[h * 128:(h + 1) * 128, :], np.float32)
    adjm = adjm.astype(ml_dtypes.bfloat16)

    in_maps = []
    ctx_of = {}   # (core, rank) -> context id
    for k in range(N_CORES):
        blob = np.zeros((128, sum(lws)), np.float32)
        for g, CT in enumerate(SCHED):
            for b, blk in enumerate(_blocks(g)):
              for ci, c in enumerate(blk):
                j = srank[g] + c
                ctx = int(order_desc[j * N_CORES + k])
                ctx_of[(k, j)] = ctx
                Gj = Gs[j]
                n = int(cnt[ctx])
                ids = order[starts[ctx]:starts[ctx] + n]
                if n == 0:
                    ids = np.zeros(1, np.int64)
                ids = ids[np.minimum(np.arange(Gj), len(ids) - 1)]
                # weights: line p, slot kk=0 -> WB rows, kk=1+hh -> WA chunks
                wcol = goff[g] + woffs[(g, b)] + ci * K * S
                blob[:, wcol:wcol + S] = WB[ctx]
                for hh in range(HS):
                    blob[:, wcol + (1 + hh) * S:wcol + (2 + hh) * S] = \
                        WA[ctx][hh * 128:(hh + 1) * 128, :]
                # activations: [A|S]-transposed sample gathers
                ao = goff[g] + aoffs[(j, 0)]
                blob[:, ao:ao + Gj] = Uf[ids].T
                XT = Xf[ids].T                          # [S, Gj]
                for hh in range(HS):
                    ao = goff[g] + aoffs[(j, 1 + hh)]
                    blob[:, ao:ao + Gj] = XT[hh * 128:(hh + 1) * 128, :]
        in_maps.append({
            "blob": blob.astype(ml_dtypes.bfloat16),
            "adjm": adjm,
        })

    if _trace:
        _install_profile_shim()
    nc = _build_program(S, A, Gs)
    res = run_bass_kernel_spmd(nc, in_maps, core_ids=list(range(N_CORES)),
                               trace=_trace)

    # device outputs: per (g, sl) a slab of R rows; context c = t*4+sl*2+cf
    # lives at free block [t, cf] of its slab.
    out_full = np.zeros((B, S), np.float32)
    for k, r in enumerate(res.results):
        vA = np.asarray(r["outA"]).astype(np.float32).reshape(-1, 2, 2, S)
        vB = np.asarray(r["outB"]).astype(np.float32).reshape(-1, 1, 2, S)
        for g, CT in enumerate(SCHED):
            for sl in range(2):
                if (g, sl) not in rowoff:
                    continue
                which, ro = rowoff[(g, sl)]
                v = vA if which == "outA" else vB
                for t in range(-(-CT // 4)):
                    for cf in range(2):
                        c = t * 4 + sl * 2 + cf
                        if c >= CT:
                            continue
                        j = srank[g] + c
                        ctx = ctx_of[(k, j)]
                        n = int(cnt[ctx])
                        if n == 0:
                            continue
                        ids = order[starts[ctx]:starts[ctx] + n]
                        out_full[ids] = v[ro:ro + n, t, cf, :]

    if _trace:
        return out_full, res
    return out_full


# revision 24
# speedup vs baseline: 1.2902x; 1.2902x over previous
"""Trainium2 Bass kernel for ContextHyperLinearSSM.

Computes out[b,:] = x[b,:] @ (WA[context[b]] * adj_xx) + u[b,:] @ (WB[context[b]] * adj_xu)

Strategy: shard the CONTEXT axis across the 8 cores.  The host sorts
contexts by sample count (descending) and deals them round-robin, so
rank j on every core has a similar count and one SPMD program (with a
per-rank padded group size G_j baked in at build time) fits all cores.
Each core streams its 64 contexts' weight banks from HBM exactly once
in bf16, applies the adjacency masks on-device, and runs 3 accumulating
matmuls per context (f32 PSUM).  Each sample's row is computed by
exactly one core, so the host-side unshard is a pure scatter.

Device-side layout: contexts are processed in groups; a group's payload
(weights + gathered/transposed x,u activations for its contexts) is
packed by the host into one contiguous HBM slab so a single DMA per
group moves 128 x ~13KB descriptors at full efficiency.  One in-place
DVE multiply per half-group masks the weights against a combined
[adjB|adjA] bf16 tile.  A group's contexts accumulate into PSUM banks
(64-aligned partition slots x 2 free halves); ACT/Pool drain the banks
with f32->bf16 cast copies.  The group schedule tapers ([8]*7+[4,2,2])
so the post-DMA pipeline tail is only a 2-context chain.
"""

import sys

sys.path.insert(0, "/opt/trn_rl_repo")

import ml_dtypes
import numpy as np

import concourse.bass as bass
import concourse.mybir as mybir
import concourse.tile as tile
from concourse import bacc
from concourse.bass_utils import run_bass_kernel_spmd

N_CORES = 8
SCHED = (8, 8, 8, 8, 8, 8, 8, 8)   # contexts per group (sum = 64)
W_BUFS = 5

# blob / matmul operand dtype: bf16 halves HBM traffic and streams the PE
# at 4x the f32 rate; PSUM still accumulates in f32 (rel err ~3e-3 << 2e-2)
IO_DT = mybir.dt.bfloat16


def _install_profile_shim():
    """Register the NTFF profile hook that trn_boot skips when
    antenv.axon_hooks is missing from the image (profiling only)."""
    import types
    if "antenv.axon_hooks" in sys.modules:
        return
    try:
        from trn_agent_boot.trn_boot import _ntff_profile_via_ctypes
        hook = _ntff_profile_via_ctypes("/opt/axon/libaxon_pjrt.so")
    except Exception:
        hook = None
    mod = types.ModuleType("antenv.axon_hooks")
    mod.get_axon_ntff_profile_hook = lambda: hook
    mod.set_axon_ntff_profile_hook = lambda h: None
    sys.modules["antenv.axon_hooks"] = mod


def _blocks(g):
    """Context blocks of group g: [W_blk | A_blk] slabs in the blob.
    Two half-blocks per group; one DMA covers the whole slab except for
    the LAST group, whose halves stream as separate DMAs so the final
    mask->matmul->drain chain overlaps the stream tail.  (Finer splits
    don't pay: DMA lines under ~6KB lose HBM efficiency.)"""
    CT = SCHED[g]
    CH = CT // 2
    return [list(range(CH)), list(range(CH, CT))]


def _layout(S, Gs):
    """Static blob/psum/output layout for per-rank group sizes Gs[64].

    Per group g the HBM slab holds, per partition line:
      [W_h0 (CH*K*S) | A_h0 (K*sum G) | W_h1 | A_h1]
    Output slabs: per (g, sl) a block of R_gsl = Gs[rank of c=sl*2] rows;
    CT=8 groups write [R, 2*2*S] rows to "outA", smaller ones [R, 2*S]
    rows to "outB".
    """
    HS = S // 128
    K = HS + 1
    NG = len(SCHED)
    srank = np.concatenate([[0], np.cumsum(SCHED)]).astype(int)
    goff, aoffs, woffs, lws = [], {}, {}, []
    col = 0
    for g, CT in enumerate(SCHED):
        goff.append(col)
        w = 0
        for b, blk in enumerate(_blocks(g)):
            woffs[(g, b)] = w
            w += len(blk) * K * S
            for c in blk:
                j = srank[g] + c
                for kk in range(K):
                    aoffs[(j, kk)] = w
                    w += Gs[j]
        lws.append(w)
        col += w
    # output row offsets per (g, sl) into outA (CT=8) / outB (CT<8)
    rowoff = {}
    rA = rB = 0
    for g, CT in enumerate(SCHED):
        for sl in range(2):
            if CT <= sl * 2:
                continue
            R = Gs[srank[g] + sl * 2]
            if CT == 8:
                rowoff[(g, sl)] = ("outA", rA)
                rA += R
            else:
                rowoff[(g, sl)] = ("outB", rB)
                rB += R
    return K, NG, srank, goff, woffs, aoffs, lws, rowoff, rA, rB


def _build_program(S, A, Gs):
    """Build the per-core Bass program. Gs = per-rank padded group sizes."""
    f32 = mybir.dt.float32
    nc = bacc.Bacc("TRN2", target_bir_lowering=False)

    assert S % 128 == 0 and A == 128
    K, NG, srank, goff, woffs, aoffs, lws, rowoff, rA, rB = _layout(S, Gs)
    LWmax = max(lws)
    TOT = sum(lws)
    FF = 2   # contexts along the free dim of a PSUM bank (512 f32 / S)

    blob = nc.dram_tensor("blob", [128, TOT], IO_DT,
                          kind="ExternalInput").ap()
    adjm = nc.dram_tensor("adjm", [128, K * S], IO_DT,
                          kind="ExternalInput").ap()
    outA = nc.dram_tensor("outA", [max(rA, 1), 2 * FF * S], IO_DT,
                          kind="ExternalOutput").ap()
    outB = nc.dram_tensor("outB", [max(rB, 1), FF * S], IO_DT,
                          kind="ExternalOutput").ap()

    with tile.TileContext(nc) as tc:
        with (
            tc.tile_pool(name="const", bufs=1) as const,
            tc.tile_pool(name="w", bufs=W_BUFS) as wpool,
            tc.tile_pool(name="o", bufs=3) as opool,
            tc.tile_pool(name="psum", bufs=8, space="PSUM") as psum,
        ):
            # combined [adjB | adjA] mask, host-prepared bf16 lines
            adjC = const.tile([128, K * S], IO_DT)
            nc.sync.dma_start(adjC[:], adjm[:])
            adjC_b = {ch: adjC[:, None, :].to_broadcast([128, ch, K * S])
                      for ch in (1, 2, 4)}

            for g, CT in enumerate(SCHED):
                NB = -(-CT // 4)   # PSUM banks for this group
                blocks = _blocks(g)
                gt = wpool.tile([128, LWmax], IO_DT, tag="gb",
                                name=f"gb_{g}")
                if g < len(SCHED) - 1:
                    # one slab DMA: widest descriptors, peak HBM efficiency
                    nc.sync.dma_start(gt[:, :lws[g]],
                                      blob[:, goff[g]:goff[g] + lws[g]])
                else:
                    # last group: per-half DMAs overlap the stream tail
                    mid = woffs[(g, 1)]
                    nc.sync.dma_start(gt[:, :mid],
                                      blob[:, goff[g]:goff[g] + mid])
                    nc.sync.dma_start(gt[:, mid:lws[g]],
                                      blob[:, goff[g] + mid:goff[g] + lws[g]])
                ps_tiles = [psum.tile([128, FF * S], f32, tag="ps",
                                      name=f"ps_{g}_{t}")
                            for t in range(NB)]
                for b, blk in enumerate(blocks):
                    woff = woffs[(g, b)]
                    # mask this block's B+A weights, in <=2-context pieces
                    # so each context's matmuls start as soon as its slice
                    # is masked (shorter DVE->PE chain at the pipeline tail)
                    PC = min(2, len(blk))
                    for pi, p0 in enumerate(range(0, len(blk), PC)):
                        wvp = gt[:, woff + p0 * K * S:
                                 woff + (p0 + PC) * K * S].rearrange(
                            "p (c ks) -> p c ks", c=PC)
                        # DVE only: Pool TT is ~4x slower (53 G elem/s)
                        nc.vector.tensor_tensor(wvp, wvp, adjC_b[PC],
                                                mybir.AluOpType.mult)
                    for ci, c in enumerate(blk):
                        j = srank[g] + c
                        Gj = Gs[j]
                        t, r2 = divmod(c, 4)
                        sl, cf = divmod(r2, 2)
                        pslice = ps_tiles[t][sl * 64:sl * 64 + Gj,
                                             cf * S:cf * S + S]
                        for kk in range(K):
                            ao = aoffs[(j, kk)]
                            wo = woff + (ci * K + kk) * S
                            nc.tensor.matmul(
                                pslice,
                                lhsT=gt[:, ao:ao + Gj],
                                rhs=gt[:, wo:wo + S],
                                start=(kk == 0), stop=(kk == K - 1))
                out_sb = opool.tile([128, NB, FF, S], IO_DT,
                                    tag=f"ob{NB}", name=f"ob_{g}")
                for t in range(NB):
                    for sl in range(2):
                        if CT <= t * 4 + sl * 2:
                            continue
                        R = Gs[srank[g] + t * 4 + sl * 2]
                        dst = out_sb[sl * 64:sl * 64 + R, t].rearrange(
                            "p f s -> p (f s)")
                        src = ps_tiles[t][sl * 64:sl * 64 + R, :]
                        # all drains on ACT: a drain on the DVE queue would
                        # make the next group's mask multiply wait behind
                        # this group's matmul completion
                        nc.scalar.copy(dst, src)
                for sl in range(2):
                    if (g, sl) not in rowoff:
                        continue
                    R = Gs[srank[g] + sl * 2]
                    which, ro = rowoff[(g, sl)]
                    dst = (outA if which == "outA" else outB)[ro:ro + R, :]
                    src = out_sb[sl * 64:sl * 64 + R].rearrange(
                        "p t f s -> p (t f s)")
                    # NOTE: issue on Scalar only — an out-DMA issue on Sync
                    # would wait for this group's drain on the Sync queue and
                    # block every later blob-DMA issue behind it.
                    nc.scalar.dma_start(dst, src)

    nc.compile()
    return nc


def kernel(x, u, WA, WB, adj_xx, adj_xu, context, _trace=False):
    B, S = x.shape
    _, A = u.shape
    C = WA.shape[0]
    assert C % N_CORES == 0
    CP = C // N_CORES
    HS = S // 128
    K = HS + 1

    # ---- host-side shard: sort contexts by count, deal round-robin ----
    context = np.asarray(context)
    cnt = np.bincount(context, minlength=C)
    order_desc = np.argsort(-cnt, kind="stable")        # context ids
    # padded group size per per-core rank j: max count among the 8 cores
    # at that rank (= the first of the slice, counts sorted desc)
    Gs = [max(2, int(-2 * (-cnt[order_desc[j * N_CORES]] // 2)))
          for j in range(CP)]
    assert max(Gs) <= 64 and sum(SCHED) == CP

    order = np.argsort(context, kind="stable")
    starts = np.zeros(C + 1, np.int64)
    starts[1:] = np.cumsum(cnt)

    Xf = np.asarray(x, np.float32)
    Uf = np.asarray(u, np.float32)
    WA = np.ascontiguousarray(WA, np.float32)
    WB = np.ascontiguousarray(WB, np.float32)

    K_, NG, srank, goff, woffs, aoffs, lws, rowoff, rA, rB = _layout(S, Gs)

    # combined mask lines: [adj_xu[p,:] | adj_xx[p,:] (chunk0) | ...]
    adjm = np.empty((128, K * S), np.float32)
    adjm[:, :S] = np.asarray(adj_xu, np.float32)
    for h in range(HS):
        adjm[:, (1 + h) * S:(2 + h) * S] = \
            np.asarray(adj_xx[h * 128:(h + 1) * 128, :], np.float32)
    adjm = adjm.astype(ml_dtypes.bfloat16)

    in_maps = []
    ctx_of = {}   # (core, rank) -> context id
    for k in range(N_CORES):
        blob = np.zeros((128, sum(lws)), np.float32)
        for g, CT in enumerate(SCHED):
            for b, blk in enumerate(_blocks(g)):
              for ci, c in enumerate(blk):
                j = srank[g] + c
                ctx = int(order_desc[j * N_CORES + k])
                ctx_of[(k, j)] = ctx
                Gj = Gs[j]
                n = int(cnt[ctx])
                ids = order[starts[ctx]:starts[ctx] + n]
                if n == 0:
                    ids = np.zeros(1, np.int64)
                ids = ids[np.minimum(np.arange(Gj), len(ids) - 1)]
                # weights: line p, slot kk=0 -> WB rows, kk=1+hh -> WA chunks
                wcol = goff[g] + woffs[(g, b)] + ci * K * S
                blob[:, wcol:wcol + S] = WB[ctx]
                for hh in range(HS):
                    blob[:, wcol + (1 + hh) * S:wcol + (2 + hh) * S] = \
                        WA[ctx][hh * 128:(hh + 1) * 128, :]
                # activations: [A|S]-transposed sample gathers
                ao = goff[g] + aoffs[(j, 0)]
                blob[:, ao:ao + Gj] = Uf[ids].T
                XT = Xf[ids].T                          # [S, Gj]
                for hh in range(HS):
                    ao = goff[g] + aoffs[(j, 1 + hh)]
                    blob[:, ao:ao + Gj] = XT[hh * 128:(hh + 1) * 128, :]
        in_maps.append({
            "blob": blob.astype(ml_dtypes.bfloat16),
            "adjm": adjm,
        })

    if _trace:
        _install_profile_shim()
    nc = _build_program(S, A, Gs)
    res = run_bass_kernel_spmd(nc, in_maps, core_ids=list(range(N_CORES)),
                               trace=_trace)

    # device outputs: per (g, sl) a slab of R rows; context c = t*4+sl*2+cf
    # lives at free block [t, cf] of its slab.
    out_full = np.zeros((B, S), np.float32)
    for k, r in enumerate(res.results):
        vA = np.asarray(r["outA"]).astype(np.float32).reshape(-1, 2, 2, S)
        vB = np.asarray(r["outB"]).astype(np.float32).reshape(-1, 1, 2, S)
        for g, CT in enumerate(SCHED):
            for sl in range(2):
                if (g, sl) not in rowoff:
                    continue
                which, ro = rowoff[(g, sl)]
                v = vA if which == "outA" else vB
                for t in range(-(-CT // 4)):
                    for cf in range(2):
                        c = t * 4 + sl * 2 + cf
                        if c >= CT:
                            continue
                        j = srank[g] + c
                        ctx = ctx_of[(k, j)]
                        n = int(cnt[ctx])
                        if n == 0:
                            continue
                        ids = order[starts[ctx]:starts[ctx] + n]
                        out_full[ids] = v[ro:ro + n, t, cf, :]

    if _trace:
        return out_full, res
    return out_full


# revision 47
# speedup vs baseline: 1.5418x; 1.1951x over previous
"""Trainium2 Bass kernel for ContextHyperLinearSSM.

Computes out[b,:] = x[b,:] @ (WA[context[b]] * adj_xx) + u[b,:] @ (WB[context[b]] * adj_xu)

Strategy: shard the CONTEXT axis across the 8 cores.  The host sorts
contexts by sample count (descending) and deals them round-robin, so
rank j on every core has a similar count and one SPMD program (with a
per-rank padded group size G_j baked in at build time) fits all cores.
Each core streams its 64 contexts' weight banks from HBM exactly once
in bf16, applies the adjacency masks on-device, and runs 3 accumulating
matmuls per context (f32 PSUM).  Each sample's row is computed by
exactly one core, so the host-side unshard is a pure scatter.

Device-side layout: contexts are processed in groups; a group's payload
(weights + gathered/transposed x,u activations for its contexts) is
packed by the host into one contiguous HBM slab so a single DMA per
group moves 128 x ~13KB descriptors at full efficiency.  One in-place
DVE multiply per half-group masks the weights against a combined
[adjB|adjA] bf16 tile.  A group's contexts accumulate into PSUM banks
(64-aligned partition slots x 2 free halves); ACT drains the banks with
f32->bf16 cast copies and issues the output DMAs.  The last group's
halves stream as two DMAs so the final mask->matmul->drain chain
overlaps the tail of the HBM stream.
"""

import sys

sys.path.insert(0, "/opt/trn_rl_repo")

import ml_dtypes
import numpy as np

import concourse.bass as bass
import concourse.mybir as mybir
import concourse.tile as tile
from concourse import bacc
from concourse.bass_utils import run_bass_kernel_spmd

N_CORES = 8
SCHED = (8, 8, 8, 8, 8, 8, 8, 8)   # contexts per group (sum = 64)
W_BUFS = 5

# blob / matmul operand dtype: bf16 halves HBM traffic and streams the PE
# at 4x the f32 rate; PSUM still accumulates in f32 (rel err ~3e-3 << 2e-2)
IO_DT = mybir.dt.bfloat16


def _install_profile_shim():
    """Register the NTFF profile hook that trn_boot skips when
    antenv.axon_hooks is missing from the image (profiling only)."""
    import types
    if "antenv.axon_hooks" in sys.modules:
        return
    try:
        from trn_agent_boot.trn_boot import _ntff_profile_via_ctypes
        hook = _ntff_profile_via_ctypes("/opt/axon/libaxon_pjrt.so")
    except Exception:
        hook = None
    mod = types.ModuleType("antenv.axon_hooks")
    mod.get_axon_ntff_profile_hook = lambda: hook
    mod.set_axon_ntff_profile_hook = lambda h: None
    sys.modules["antenv.axon_hooks"] = mod


def _blocks(g):
    """Context blocks of group g: [W_blk | A_blk] slabs in the blob.
    Two half-blocks per group; one DMA covers the whole slab except for
    the FIRST group (halves start the DVE mask stream earlier) and the
    LAST (halves let the final mask->matmul->drain chain overlap the
    stream tail).  Finer splits don't pay: DMA lines under ~6KB lose
    HBM efficiency."""
    CT = SCHED[g]
    CH = CT // 2
    return [list(range(CH)), list(range(CH, CT))]


def _layout(S, Gs):
    """Static blob/psum/output layout for per-rank group sizes Gs[64].

    Per group g the HBM slab holds, per partition line:
      [W_h0 (CH*K*S) | A_h0 (K*sum G) | W_h1 | A_h1]
    Output slabs: per (g, sl) a block of R_gsl = Gs[rank of c=sl*2] rows;
    CT=8 groups write [R, 2*2*S] rows to "outA", smaller ones [R, 2*S]
    rows to "outB".
    """
    HS = S // 128
    K = HS + 1
    NG = len(SCHED)
    srank = np.concatenate([[0], np.cumsum(SCHED)]).astype(int)
    goff, aoffs, woffs, lws = [], {}, {}, []
    col = 0
    for g, CT in enumerate(SCHED):
        goff.append(col)
        w = 0
        for b, blk in enumerate(_blocks(g)):
            woffs[(g, b)] = w
            w += len(blk) * K * S
            for c in blk:
                j = srank[g] + c
                for kk in range(K):
                    aoffs[(j, kk)] = w
                    w += Gs[j]
        lws.append(w)
        col += w
    # output row offsets per (g, sl) into outA (CT=8) / outB (CT<8)
    rowoff = {}
    rA = rB = 0
    for g, CT in enumerate(SCHED):
        for sl in range(2):
            if CT <= sl * 2:
                continue
            R = Gs[srank[g] + sl * 2]
            if CT == 8:
                rowoff[(g, sl)] = ("outA", rA)
                rA += R
            else:
                rowoff[(g, sl)] = ("outB", rB)
                rB += R
    return K, NG, srank, goff, woffs, aoffs, lws, rowoff, rA, rB


def _build_program(S, A, Gs):
    """Build the per-core Bass program. Gs = per-rank padded group sizes."""
    f32 = mybir.dt.float32
    nc = bacc.Bacc("TRN2", target_bir_lowering=False)

    assert S % 128 == 0 and A == 128
    K, NG, srank, goff, woffs, aoffs, lws, rowoff, rA, rB = _layout(S, Gs)
    LWmax = max(lws)
    TOT = sum(lws)
    FF = 2   # contexts along the free dim of a PSUM bank (512 f32 / S)

    blob = nc.dram_tensor("blob", [128, TOT], IO_DT,
                          kind="ExternalInput").ap()
    adjm = nc.dram_tensor("adjm", [128, K * S], IO_DT,
                          kind="ExternalInput").ap()
    outA = nc.dram_tensor("outA", [max(rA, 1), 2 * FF * S], IO_DT,
                          kind="ExternalOutput").ap()
    outB = nc.dram_tensor("outB", [max(rB, 1), FF * S], IO_DT,
                          kind="ExternalOutput").ap()

    with tile.TileContext(nc) as tc:
        with (
            tc.tile_pool(name="const", bufs=1) as const,
            tc.tile_pool(name="w", bufs=W_BUFS) as wpool,
            tc.tile_pool(name="o", bufs=3) as opool,
            tc.tile_pool(name="psum", bufs=8, space="PSUM") as psum,
        ):
            # combined [adjB | adjA] mask, host-prepared bf16 lines (a u8
            # operand would put the DVE TT on a 2x-slower mixed-dtype
            # path), replicated to 2 contexts for flat 2D TT APs.  Issued
            # on the Scalar HWDGE queue: keeps the Sync queue free so the
            # first blob DMA issues earlier.
            adjC = const.tile([128, 2, K * S], IO_DT)
            nc.scalar.dma_start(adjC[:, 0], adjm[:])
            nc.scalar.copy(adjC[:, 1], adjC[:, 0])
            adjR = adjC.rearrange("p c ks -> p (c ks)")

            for g, CT in enumerate(SCHED):
                NB = -(-CT // 4)   # PSUM banks for this group
                blocks = _blocks(g)
                gt = wpool.tile([128, LWmax], IO_DT, tag="gb",
                                name=f"gb_{g}")
                if 0 < g < len(SCHED) - 1:
                    # one slab DMA: widest descriptors, peak HBM efficiency
                    nc.sync.dma_start(gt[:, :lws[g]],
                                      blob[:, goff[g]:goff[g] + lws[g]])
                else:
                    # first group: per-half DMAs start the DVE mask stream
                    # earlier; last group: they overlap the stream tail
                    mid = woffs[(g, 1)]
                    nc.sync.dma_start(gt[:, :mid],
                                      blob[:, goff[g]:goff[g] + mid])
                    nc.sync.dma_start(gt[:, mid:lws[g]],
                                      blob[:, goff[g] + mid:goff[g] + lws[g]])
                ps_tiles = [psum.tile([128, FF * S], f32, tag="ps",
                                      name=f"ps_{g}_{t}")
                            for t in range(NB)]
                for b, blk in enumerate(blocks):
                    woff = woffs[(g, b)]
                    # mask this block's B+A weights in place (DVE TT mult
                    # is SBUF-bandwidth-bound at ~207 G elem/s; in-place
                    # vs separate dst and flat vs broadcast APs measure
                    # identical), in <=2-context pieces so each context's
                    # matmuls start as soon as its slice is masked.  DVE
                    # only: Pool TT is both 4x slower and shares the queue
                    # with the Tile framework's semaphore bookkeeping, so
                    # real work there stalls the whole pipeline.
                    PC = min(2, len(blk))
                    for pi, p0 in enumerate(range(0, len(blk), PC)):
                        wvp = gt[:, woff + p0 * K * S:
                                 woff + (p0 + PC) * K * S]
                        nc.vector.tensor_tensor(wvp, wvp,
                                                adjR[:, :PC * K * S],
                                                mybir.AluOpType.mult)
                    for ci, c in enumerate(blk):
                        j = srank[g] + c
                        Gj = Gs[j]
                        t, r2 = divmod(c, 4)
                        sl, cf = divmod(r2, 2)
                        pslice = ps_tiles[t][sl * 64:sl * 64 + Gj,
                                             cf * S:cf * S + S]
                        for kk in range(K):
                            ao = aoffs[(j, kk)]
                            wo = woff + (ci * K + kk) * S
                            nc.tensor.matmul(
                                pslice,
                                lhsT=gt[:, ao:ao + Gj],
                                rhs=gt[:, wo:wo + S],
                                start=(kk == 0), stop=(kk == K - 1))
                out_sb = opool.tile([128, NB, FF, S], IO_DT,
                                    tag=f"ob{NB}", name=f"ob_{g}")
                for t in range(NB):
                    for sl in range(2):
                        if CT <= t * 4 + sl * 2:
                            continue
                        R = Gs[srank[g] + t * 4 + sl * 2]
                        dst = out_sb[sl * 64:sl * 64 + R, t].rearrange(
                            "p f s -> p (f s)")
                        src = ps_tiles[t][sl * 64:sl * 64 + R, :]
                        # drains on ACT: a drain on the DVE queue would make
                        # the next group's mask multiply wait behind this
                        # group's matmuls.  The LAST group has no successor,
                        # so its sl=1 drains run on DVE in parallel.
                        if g == len(SCHED) - 1 and sl == 1:
                            nc.vector.tensor_copy(dst, src)
                        else:
                            nc.scalar.copy(dst, src)
                for sl in range(2):
                    if (g, sl) not in rowoff:
                        continue
                    R = Gs[srank[g] + sl * 2]
                    which, ro = rowoff[(g, sl)]
                    dst = (outA if which == "outA" else outB)[ro:ro + R, :]
                    src = out_sb[sl * 64:sl * 64 + R].rearrange(
                        "p t f s -> p (t f s)")
                    # NOTE: issue on Scalar only — an out-DMA issue on Sync
                    # would wait for this group's drain on the Sync queue and
                    # block every later blob-DMA issue behind it.
                    nc.scalar.dma_start(dst, src)

    nc.compile()
    return nc


def kernel(x, u, WA, WB, adj_xx, adj_xu, context, _trace=False):
    B, S = x.shape
    _, A = u.shape
    C = WA.shape[0]
    assert C % N_CORES == 0
    CP = C // N_CORES
    HS = S // 128
    K = HS + 1

    # ---- host-side shard: sort contexts by count, deal round-robin ----
    context = np.asarray(context)
    cnt = np.bincount(context, minlength=C)
    order_desc = np.argsort(-cnt, kind="stable")        # context ids
    # padded group size per per-core rank j: max count among the 8 cores
    # at that rank (= the first of the slice, counts sorted desc)
    Gs = [max(2, int(-2 * (-cnt[order_desc[j * N_CORES]] // 2)))
          for j in range(CP)]
    assert max(Gs) <= 64 and sum(SCHED) == CP

    order = np.argsort(context, kind="stable")
    starts = np.zeros(C + 1, np.int64)
    starts[1:] = np.cumsum(cnt)

    Xf = np.asarray(x, np.float32)
    Uf = np.asarray(u, np.float32)
    WA = np.ascontiguousarray(WA, np.float32)
    WB = np.ascontiguousarray(WB, np.float32)

    K_, NG, srank, goff, woffs, aoffs, lws, rowoff, rA, rB = _layout(S, Gs)

    # combined mask lines: [adj_xu[p,:] | adj_xx[p,:] (chunk0) | ...]
    adjm = np.empty((128, K * S), np.float32)
    adjm[:, :S] = np.asarray(adj_xu, np.float32)
    for h in range(HS):
        adjm[:, (1 + h) * S:(2 + h) * S] = \
            np.asarray(adj_xx[h * 128:(h + 1) * 128, :], np.float32)
    adjm = adjm.astype(ml_dtypes.bfloat16)

    in_maps = []
    ctx_of = {}   # (core, rank) -> context id
    for k in range(N_CORES):
        blob = np.zeros((128, sum(lws)), np.float32)
        for g, CT in enumerate(SCHED):
            for b, blk in enumerate(_blocks(g)):
              for ci, c in enumerate(blk):
                j = srank[g] + c
                ctx = int(order_desc[j * N_CORES + k])
                ctx_of[(k, j)] = ctx
                Gj = Gs[j]
                n = int(cnt[ctx])
                ids = order[starts[ctx]:starts[ctx] + n]
                if n == 0:
                    ids = np.zeros(1, np.int64)
                ids = ids[np.minimum(np.arange(Gj), len(ids) - 1)]
                # weights: line p, slot kk=0 -> WB rows, kk=1+hh -> WA chunks
                wcol = goff[g] + woffs[(g, b)] + ci * K * S
                blob[:, wcol:wcol + S] = WB[ctx]
                for hh in range(HS):
                    blob[:, wcol + (1 + hh) * S:wcol + (2 + hh) * S] = \
                        WA[ctx][hh * 128:(hh + 1) * 128, :]
                # activations: [A|S]-transposed sample gathers
                ao = goff[g] + aoffs[(j, 0)]
                blob[:, ao:ao + Gj] = Uf[ids].T
                XT = Xf[ids].T                          # [S, Gj]
                for hh in range(HS):
                    ao = goff[g] + aoffs[(j, 1 + hh)]
                    blob[:, ao:ao + Gj] = XT[hh * 128:(hh + 1) * 128, :]
        in_maps.append({
            "blob": blob.astype(ml_dtypes.bfloat16),
            "adjm": adjm,
        })

    if _trace:
        _install_profile_shim()
    nc = _build_program(S, A, Gs)
    res = run_bass_kernel_spmd(nc, in_maps, core_ids=list(range(N_CORES)),
                               trace=_trace)

    # device outputs: per (g, sl) a slab of R rows; context c = t*4+sl*2+cf
    # lives at free block [t, cf] of its slab.
    out_full = np.zeros((B, S), np.float32)
    for k, r in enumerate(res.results):
        vA = np.asarray(r["outA"]).astype(np.float32).reshape(-1, 2, 2, S)
        vB = np.asarray(r["outB"]).astype(np.float32).reshape(-1, 1, 2, S)
        for g, CT in enumerate(SCHED):
            for sl in range(2):
                if (g, sl) not in rowoff:
                    continue
                which, ro = rowoff[(g, sl)]
                v = vA if which == "outA" else vB
                for t in range(-(-CT // 4)):
                    for cf in range(2):
                        c = t * 4 + sl * 2 + cf
                        if c >= CT:
                            continue
                        j = srank[g] + c
                        ctx = ctx_of[(k, j)]
                        n = int(cnt[ctx])
                        if n == 0:
                            continue
                        ids = order[starts[ctx]:starts[ctx] + n]
                        out_full[ids] = v[ro:ro + n, t, cf, :]

    if _trace:
        return out_full, res
    return out_full
